# revision 3
# baseline (speedup 1.0000x reference)
"""Bass/Trainium2 kernel for nn_EquivariantProductBasisBlock.

Math (per node n, feature f):
    s = x[n,f,0]; v = x[n,f,1:4]; vv = (v.v)/sqrt(3)
    out0 = a0[sp,0]*s + a0[sp,1]*s^2 + a0[sp,2]*vv + a0[sp,3]*s^3 + a0[sp,4]*(s*vv)
    c1   = a1[sp,0] + a1[sp,1]*s + a1[sp,2]*s^2 + a1[sp,3]*vv
    y0 = out0 @ W0 / 16 ;  y1_c = (c1*v_c) @ W1 / 16
    out = concat(y0, y1) over the lm axis.

Strategy: shard nodes over 8 cores. Host sorts nodes by species so the
species-dependent path weights become per-partition scalar columns on
device (features on partitions, nodes on the free axis). The 1/sqrt(3)
and 1/16 factors are folded into the weight tables on the host.
"""

import numpy as np
from contextlib import ExitStack

N_CORES = 8
F = 256
NUM_SPECIES = 10
NB = 1024   # nodes per compute block
SUB = 512   # nodes per matmul / store sub-block (one PSUM bank fp32)
INV_SQRT3 = 1.0 / np.sqrt(3.0)
INV_SQRT_F = 1.0 / np.sqrt(256.0)

_KERNEL_CACHE = {}


def _build_bass(c_sp, ntot):
    """Build + compile the per-core Bass graph.

    c_sp: per-species padded segment length (same on every core).
    ntot: total padded nodes per core (multiple of SUB).
    """
    import concourse.bacc as bacc
    import concourse.mybir as mybir
    import concourse.tile as tile

    fp32 = mybir.dt.float32
    AF = mybir.ActivationFunctionType
    OP = mybir.AluOpType

    nc = bacc.Bacc("TRN2", target_bir_lowering=False, debug=False)

    x = nc.dram_tensor("x", [8, 128, ntot], fp32, kind="ExternalInput")
    a0 = nc.dram_tensor("a0", [256, 5 * NUM_SPECIES], fp32, kind="ExternalInput")
    a1 = nc.dram_tensor("a1", [256, 4 * NUM_SPECIES], fp32, kind="ExternalInput")
    w0 = nc.dram_tensor("w0", [256, 256], fp32, kind="ExternalInput")
    w1 = nc.dram_tensor("w1", [256, 256], fp32, kind="ExternalInput")
    y = nc.dram_tensor("y", [8, 128, ntot], fp32, kind="ExternalOutput")

    xr = x[:].rearrange("s p n -> p s n")
    yr = y[:].rearrange("s p n -> p s n")

    # node blocks
    blocks = []
    j = 0
    while j < ntot:
        nb = min(NB, ntot - j)
        blocks.append((j, nb))
        j += nb

    # species segment [lo, hi) boundaries in the padded node axis
    ends = np.cumsum(c_sp)

    def segments(j0, nb):
        segs = []
        for sp in range(NUM_SPECIES):
            lo = int(ends[sp] - c_sp[sp])
            hi = int(ends[sp])
            a = max(lo, j0)
            b = min(hi, j0 + nb)
            if a < b:
                segs.append((sp, a - j0, b - a))
        return segs

    with tile.TileContext(nc) as tc:
        with ExitStack() as ctx:
            consts = ctx.enter_context(tc.tile_pool(name="consts", bufs=1))
            io_in = ctx.enter_context(tc.tile_pool(name="io_in", bufs=2))
            rhs_p = ctx.enter_context(tc.tile_pool(name="rhs", bufs=2))
            tmp = ctx.enter_context(tc.tile_pool(name="tmp", bufs=1))
            stag = ctx.enter_context(tc.tile_pool(name="stag", bufs=2))
            psum = ctx.enter_context(tc.tile_pool(name="psum", bufs=8, space="PSUM"))

            # --- constants ---
            w0_sb = consts.tile([128, 2, 256], fp32)
            nc.sync.dma_start(out=w0_sb, in_=w0[:].rearrange("(fc p) g -> p fc g", p=128))
            w1_sb = consts.tile([128, 2, 256], fp32)
            nc.sync.dma_start(out=w1_sb, in_=w1[:].rearrange("(fc p) g -> p fc g", p=128))
            a0_sb = consts.tile([128, 2, 5 * NUM_SPECIES], fp32)
            nc.sync.dma_start(out=a0_sb, in_=a0[:].rearrange("(fc p) c -> p fc c", p=128))
            a1_sb = consts.tile([128, 2, 4 * NUM_SPECIES], fp32)
            nc.sync.dma_start(out=a1_sb, in_=a1[:].rearrange("(fc p) c -> p fc c", p=128))

            def a0c(fc, sp, p):
                i = sp * 5 + p
                return a0_sb[:, fc, i : i + 1]

            def a1c(fc, sp, p):
                i = sp * 4 + p
                return a1_sb[:, fc, i : i + 1]

            for (j0, nb) in blocks:
                segs = segments(j0, nb)
                nsub = nb // SUB

                xin = io_in.tile([128, 8, nb], fp32, tag="xin")
                nc.sync.dma_start(out=xin, in_=xr[:, :, j0 : j0 + nb])

                # rhs[comp] : [128, fc, nb] GEMM moving operands
                rhs = [rhs_p.tile([128, 2, nb], fp32, tag=f"rhs{c}", name=f"rhs{c}_{j0}") for c in range(4)]

                for fc in range(2):
                    s_ = xin[:, fc, :]
                    vx = xin[:, 2 + fc, :]
                    vy = xin[:, 4 + fc, :]
                    vz = xin[:, 6 + fc, :]

                    q = tmp.tile([128, nb], fp32, tag="q")
                    ta = tmp.tile([128, nb], fp32, tag="ta")
                    tb = tmp.tile([128, nb], fp32, tag="tb")
                    td = tmp.tile([128, nb], fp32, tag="td")
                    vv = tmp.tile([128, nb], fp32, tag="vv")
                    h = tmp.tile([128, nb], fp32, tag="h")
                    z = tmp.tile([128, nb], fp32, tag="z")
                    u = tmp.tile([128, nb], fp32, tag="u")
                    k = tmp.tile([128, nb], fp32, tag="k")

                    # squares on ScalarE
                    nc.scalar.activation(q, s_, AF.Square)
                    nc.scalar.activation(ta, vx, AF.Square)
                    nc.scalar.activation(tb, vy, AF.Square)
                    # vv = vx^2 + vy^2 + vz^2 (adds on GPSIMD)
                    nc.gpsimd.tensor_tensor(td, ta, tb, OP.add)
                    tc2 = tmp.tile([128, nb], fp32, tag="ta")
                    nc.scalar.activation(tc2, vz, AF.Square)
                    nc.gpsimd.tensor_tensor(vv, td, tc2, OP.add)

                    # h = a0[3]*s + a0[1]   (per-species, ScalarE affine)
                    for (sp, o, L) in segs:
                        nc.scalar.activation(
                            h[:, o : o + L], s_[:, o : o + L], AF.Identity,
                            bias=a0c(fc, sp, 1), scale=a0c(fc, sp, 3),
                        )
                    # z = q * h = a0[1]*s^2 + a0[3]*s^3
                    nc.vector.tensor_tensor(z, q, h, OP.mult)
                    # u = s * vv
                    nc.vector.tensor_tensor(u, s_, vv, OP.mult)

                    # k = a1[1]*s + a1[0]
                    for (sp, o, L) in segs:
                        nc.vector.tensor_scalar(
                            k[:, o : o + L], s_[:, o : o + L],
                            a1c(fc, sp, 1), a1c(fc, sp, 0), OP.mult, OP.add,
                        )

                    # acc = a0[0]*s + z   (reuse h slot)
                    acc = tmp.tile([128, nb], fp32, tag="h")
                    for (sp, o, L) in segs:
                        nc.vector.scalar_tensor_tensor(
                            acc[:, o : o + L], s_[:, o : o + L],
                            a0c(fc, sp, 0), z[:, o : o + L], OP.mult, OP.add,
                        )
                    # acc2 = a0[2]*vv + acc  (reuse z slot)
                    acc2 = tmp.tile([128, nb], fp32, tag="z")
                    for (sp, o, L) in segs:
                        nc.vector.scalar_tensor_tensor(
                            acc2[:, o : o + L], vv[:, o : o + L],
                            a0c(fc, sp, 2), acc[:, o : o + L], OP.mult, OP.add,
                        )
                    # out0 = a0[4]*u + acc2  -> rhs[0]
                    for (sp, o, L) in segs:
                        nc.vector.scalar_tensor_tensor(
                            rhs[0][:, fc, o : o + L], u[:, o : o + L],
                            a0c(fc, sp, 4), acc2[:, o : o + L], OP.mult, OP.add,
                        )

                    # k2 = a1[2]*q + k  (reuse tb slot)
                    k2 = tmp.tile([128, nb], fp32, tag="tb")
                    for (sp, o, L) in segs:
                        nc.vector.scalar_tensor_tensor(
                            k2[:, o : o + L], q[:, o : o + L],
                            a1c(fc, sp, 2), k[:, o : o + L], OP.mult, OP.add,
                        )
                    # c1 = a1[3]*vv + k2  (reuse k slot)
                    c1 = tmp.tile([128, nb], fp32, tag="k")
                    for (sp, o, L) in segs:
                        nc.vector.scalar_tensor_tensor(
                            c1[:, o : o + L], vv[:, o : o + L],
                            a1c(fc, sp, 3), k2[:, o : o + L], OP.mult, OP.add,
                        )

                    # out1 = c1 * v
                    nc.vector.tensor_tensor(rhs[1][:, fc, :], c1, vx, OP.mult)
                    nc.gpsimd.tensor_tensor(rhs[2][:, fc, :], c1, vy, OP.mult)
                    nc.gpsimd.tensor_tensor(rhs[3][:, fc, :], c1, vz, OP.mult)

                # --- GEMM: y[comp] = rhs[comp] @ W (K=256 over fc chunks) ---
                stg = [stag.tile([128, 8, SUB], fp32, tag="stg", name=f"stg{si}_{j0}") for si in range(nsub)]
                ps = {}
                for comp in range(4):
                    for gc in range(2):
                        for si in range(nsub):
                            ps[(comp, gc, si)] = psum.tile([128, SUB], fp32, tag="ps", name=f"ps{comp}{gc}{si}_{j0}")

                for gc in range(2):
                    g0 = gc * 128
                    for fc in range(2):
                        for comp in range(4):
                            w_sb = w0_sb if comp == 0 else w1_sb
                            lhsT = w_sb[:, fc, g0 : g0 + 128]
                            for si in range(nsub):
                                o = si * SUB
                                nc.tensor.matmul(
                                    ps[(comp, gc, si)],
                                    lhsT,
                                    rhs[comp][:, fc, o : o + SUB],
                                    start=(fc == 0),
                                    stop=(fc == 1),
                                )
                                if fc == 1:
                                    nc.scalar.activation(
                                        stg[si][:, comp * 2 + gc, :],
                                        ps[(comp, gc, si)],
                                        AF.Copy,
                                    )

                for si in range(nsub):
                    o = j0 + si * SUB
                    nc.sync.dma_start(out=yr[:, :, o : o + SUB], in_=stg[si])

    nc.compile()
    return nc


def _prepare(node_feats, node_specie, w0, w1, W0, W1):
    """Host-side: sort by species, shard, transpose, fold scale factors."""
    n = node_feats.shape[0]
    sp = np.asarray(node_specie).astype(np.int64)

    # per-core, per-species node id lists (round-robin for balance)
    ids_by_sp = [np.nonzero(sp == s)[0] for s in range(NUM_SPECIES)]
    core_ids = [[ids_by_sp[s][c::N_CORES] for s in range(NUM_SPECIES)] for c in range(N_CORES)]
    c_sp = [max(len(core_ids[c][s]) for c in range(N_CORES)) for s in range(NUM_SPECIES)]
    ntot = int(np.sum(c_sp))
    pad_tail = (-ntot) % SUB
    c_sp[-1] += pad_tail
    ntot += pad_tail

    # per-core padded index arrays + valid masks
    idx = np.zeros((N_CORES, ntot), dtype=np.int64)
    valid = np.zeros((N_CORES, ntot), dtype=bool)
    off = 0
    for s in range(NUM_SPECIES):
        L = c_sp[s]
        for c in range(N_CORES):
            ids = core_ids[c][s]
            k = len(ids)
            idx[c, off : off + k] = ids
            valid[c, off : off + k] = True
            # pads: reuse node 0 (values computed then discarded)
        off += L

    # weight tables with folded normalization
    w0a = np.asarray(w0, np.float32).copy()
    w1a = np.asarray(w1, np.float32).copy()
    w0a[:, 2, :] *= INV_SQRT3
    w0a[:, 4, :] *= INV_SQRT3
    w1a[:, 3, :] *= INV_SQRT3
    a0_tab = np.ascontiguousarray(w0a.transpose(2, 0, 1).reshape(F, 5 * NUM_SPECIES))
    a1_tab = np.ascontiguousarray(w1a.transpose(2, 0, 1).reshape(F, 4 * NUM_SPECIES))
    W0s = np.ascontiguousarray(np.asarray(W0, np.float32) * INV_SQRT_F)
    W1s = np.ascontiguousarray(np.asarray(W1, np.float32) * INV_SQRT_F)

    # input marshaling: [n, 256, 4] -> per core [8, 128, ntot]
    xt = np.ascontiguousarray(np.asarray(node_feats, np.float32).transpose(2, 1, 0))  # [4,256,n]
    xs = []
    for c in range(N_CORES):
        xc = xt[:, :, idx[c]]  # [4, 256, ntot]
        xs.append(np.ascontiguousarray(xc.reshape(8, 128, ntot)))

    return xs, idx, valid, tuple(c_sp), ntot, a0_tab, a1_tab, W0s, W1s


def kernel(node_feats, node_specie, w0, w1, W0, W1):
    from concourse.bass_utils import run_bass_kernel_spmd

    xs, idx, valid, c_sp, ntot, a0_tab, a1_tab, W0s, W1s = _prepare(
        node_feats, node_specie, w0, w1, W0, W1
    )

    key = (c_sp, ntot)
    if key not in _KERNEL_CACHE:
        _KERNEL_CACHE[key] = _build_bass(list(c_sp), ntot)
    nc = _KERNEL_CACHE[key]

    in_maps = [
        {"x": xs[c], "a0": a0_tab, "a1": a1_tab, "w0": W0s, "w1": W1s}
        for c in range(N_CORES)
    ]
    res = run_bass_kernel_spmd(nc, in_maps, core_ids=list(range(N_CORES)))

    n = node_feats.shape[0]
    out = np.empty((n, F, 4), dtype=np.float32)
    for c in range(N_CORES):
        yc = res.results[c]["y"].reshape(4, F, ntot)
        yt = np.ascontiguousarray(yc.transpose(2, 1, 0))  # [ntot, 256, 4]
        m = valid[c]
        out[idx[c][m]] = yt[m]
    return out


# revision 5
# speedup vs baseline: 1.5902x; 1.5902x over previous
"""Bass/Trainium2 kernel for nn_EquivariantProductBasisBlock.

Math (per node n, feature f):
    s = x[n,f,0]; v = x[n,f,1:4]; vv = (v.v)/sqrt(3)
    out0 = a0[sp,0]*s + a0[sp,1]*s^2 + a0[sp,2]*vv + a0[sp,3]*s^3 + a0[sp,4]*(s*vv)
    c1   = a1[sp,0] + a1[sp,1]*s + a1[sp,2]*s^2 + a1[sp,3]*vv
    y0 = out0 @ W0 / 16 ;  y1_c = (c1*v_c) @ W1 / 16
    out = concat(y0, y1) over the lm axis.

Strategy: shard nodes over 8 cores. Host sorts nodes by species so the
species-dependent path weights become per-partition scalar columns on
device (features on partitions, nodes on the free axis). The 1/sqrt(3)
and 1/16 factors are folded into the weight tables on the host.
Elementwise + GEMM run in bf16 (fp32 PSUM accumulation); I/O is bf16
with the final output upcast on the host.

Polynomial factorization (all per-species coefficients fold into
ScalarE affines or tensor_scalar per-partition scalars):
    h2 = a3*s + a1           (ACT affine, per species)
    B  = a4*s + a2           (ACT affine)
    gg = a12*s + a11         (ACT affine)
    out0 = s*(s*h2 + a0) + vv*B
    c1   = s*gg + (a13*vv + a10)
"""

import numpy as np
from contextlib import ExitStack

import ml_dtypes

N_CORES = 8
F = 256
NUM_SPECIES = 10
NB = 1024   # nodes per compute block
SUB = 512   # nodes per matmul / store sub-block (one PSUM bank fp32)
INV_SQRT3 = 1.0 / np.sqrt(3.0)
INV_SQRT_F = 1.0 / np.sqrt(256.0)

_KERNEL_CACHE = {}


def _build_bass(c_sp, ntot):
    """Build + compile the per-core Bass graph.

    c_sp: per-species padded segment length (same on every core), even.
    ntot: total padded nodes per core (multiple of SUB).
    """
    import concourse.bacc as bacc
    import concourse.mybir as mybir
    import concourse.tile as tile

    fp32 = mybir.dt.float32
    bf16 = mybir.dt.bfloat16
    AF = mybir.ActivationFunctionType
    OP = mybir.AluOpType

    nc = bacc.Bacc("TRN2", target_bir_lowering=False, debug=False)

    x = nc.dram_tensor("x", [8, 128, ntot], bf16, kind="ExternalInput")
    a0 = nc.dram_tensor("a0", [256, 5 * NUM_SPECIES], fp32, kind="ExternalInput")
    a1 = nc.dram_tensor("a1", [256, 4 * NUM_SPECIES], fp32, kind="ExternalInput")
    w0 = nc.dram_tensor("w0", [256, 256], bf16, kind="ExternalInput")
    w1 = nc.dram_tensor("w1", [256, 256], bf16, kind="ExternalInput")
    y = nc.dram_tensor("y", [8, 128, ntot], bf16, kind="ExternalOutput")

    xr = x[:].rearrange("s p n -> p s n")
    yr = y[:].rearrange("s p n -> p s n")

    # node blocks
    blocks = []
    j = 0
    while j < ntot:
        nb = min(NB, ntot - j)
        blocks.append((j, nb))
        j += nb

    ends = np.cumsum(c_sp)

    def segments(j0, nb):
        segs = []
        for sp in range(NUM_SPECIES):
            lo = int(ends[sp] - c_sp[sp])
            hi = int(ends[sp])
            a = max(lo, j0)
            b = min(hi, j0 + nb)
            if a < b:
                segs.append((sp, a - j0, b - a))
        return segs

    with tile.TileContext(nc) as tc:
        with ExitStack() as ctx:
            consts = ctx.enter_context(tc.tile_pool(name="consts", bufs=1))
            io_in = ctx.enter_context(tc.tile_pool(name="io_in", bufs=3))
            rhs_p = ctx.enter_context(tc.tile_pool(name="rhs", bufs=2))
            tmp = ctx.enter_context(tc.tile_pool(name="tmp", bufs=2))
            stag = ctx.enter_context(tc.tile_pool(name="stag", bufs=3))
            psum = ctx.enter_context(tc.tile_pool(name="psum", bufs=8, space="PSUM"))

            # --- constants ---
            w0_sb = consts.tile([128, 2, 256], bf16)
            nc.sync.dma_start(out=w0_sb, in_=w0[:].rearrange("(fc p) g -> p fc g", p=128))
            w1_sb = consts.tile([128, 2, 256], bf16)
            nc.sync.dma_start(out=w1_sb, in_=w1[:].rearrange("(fc p) g -> p fc g", p=128))
            a0_sb = consts.tile([128, 2, 5 * NUM_SPECIES], fp32)
            nc.sync.dma_start(out=a0_sb, in_=a0[:].rearrange("(fc p) c -> p fc c", p=128))
            a1_sb = consts.tile([128, 2, 4 * NUM_SPECIES], fp32)
            nc.sync.dma_start(out=a1_sb, in_=a1[:].rearrange("(fc p) c -> p fc c", p=128))

            def a0c(fc, sp, p):
                i = sp * 5 + p
                return a0_sb[:, fc, i : i + 1]

            def a1c(fc, sp, p):
                i = sp * 4 + p
                return a1_sb[:, fc, i : i + 1]

            for (j0, nb) in blocks:
                segs = segments(j0, nb)
                nsub = nb // SUB

                xin = io_in.tile([128, 8, nb], bf16, tag="xin", name=f"xin_{j0}")
                nc.sync.dma_start(out=xin, in_=xr[:, :, j0 : j0 + nb])

                rhs = [rhs_p.tile([128, 2, nb], bf16, tag=f"rhs{c}", name=f"rhs{c}_{j0}") for c in range(4)]

                for fc in range(2):
                    s_ = xin[:, fc, :]
                    vx = xin[:, 2 + fc, :]
                    vy = xin[:, 4 + fc, :]
                    vz = xin[:, 6 + fc, :]

                    vvx = tmp.tile([128, nb], bf16, tag="vvx")
                    vvy = tmp.tile([128, nb], bf16, tag="vvy")
                    vvz = tmp.tile([128, nb], bf16, tag="vvz")
                    ad = tmp.tile([128, nb], bf16, tag="ad")
                    vv = tmp.tile([128, nb], bf16, tag="vv")
                    h2 = tmp.tile([128, nb], bf16, tag="h2")
                    bb = tmp.tile([128, nb], bf16, tag="bb")
                    gg = tmp.tile([128, nb], bf16, tag="gg")
                    h3 = tmp.tile([128, nb], bf16, tag="h3")
                    tt = tmp.tile([128, nb], bf16, tag="tt")
                    p1 = tmp.tile([128, nb], bf16, tag="p1")
                    w_ = tmp.tile([128, nb], bf16, tag="w_")

                    # vv = vx^2 + vy^2 + vz^2 (INV_SQRT3 folded into tables)
                    nc.vector.tensor_tensor(vvx, vx, vx, OP.mult)
                    nc.vector.tensor_tensor(vvy, vy, vy, OP.mult)
                    nc.gpsimd.tensor_tensor(vvz, vz, vz, OP.mult)
                    nc.vector.tensor_tensor(ad, vvx, vvy, OP.add)
                    nc.vector.tensor_tensor(vv, ad, vvz, OP.add)

                    # per-species affines on ScalarE
                    for (sp, o, L) in segs:
                        sl = slice(o, o + L)
                        nc.scalar.activation(h2[:, sl], s_[:, sl], AF.Identity,
                                             bias=a0c(fc, sp, 1), scale=a0c(fc, sp, 3))
                        nc.scalar.activation(bb[:, sl], s_[:, sl], AF.Identity,
                                             bias=a0c(fc, sp, 2), scale=a0c(fc, sp, 4))
                        nc.scalar.activation(gg[:, sl], s_[:, sl], AF.Identity,
                                             bias=a1c(fc, sp, 1), scale=a1c(fc, sp, 2))

                    # out0 = s*(s*h2 + a0[0]) + vv*B
                    nc.vector.tensor_tensor(h3, s_, h2, OP.mult)
                    h4 = tmp.tile([128, nb], bf16, tag="h2")
                    for (sp, o, L) in segs:
                        nc.vector.tensor_scalar(
                            h4[:, o : o + L], h3[:, o : o + L],
                            a0c(fc, sp, 0), None, OP.add,
                        )
                    nc.vector.tensor_tensor(tt, vv, bb, OP.mult)
                    p0 = tmp.tile([128, nb], bf16, tag="h3")
                    nc.vector.tensor_tensor(p0, s_, h4, OP.mult)
                    nc.vector.tensor_tensor(rhs[0][:, fc, :], p0, tt, OP.add)

                    # c1 = s*gg + (a13*vv + a10)
                    nc.vector.tensor_tensor(p1, s_, gg, OP.mult)
                    for (sp, o, L) in segs:
                        nc.vector.tensor_scalar(
                            w_[:, o : o + L], vv[:, o : o + L],
                            a1c(fc, sp, 3), a1c(fc, sp, 0), OP.mult, OP.add,
                        )
                    c1 = tmp.tile([128, nb], bf16, tag="gg")
                    nc.vector.tensor_tensor(c1, p1, w_, OP.add)

                    # out1 = c1 * v
                    nc.vector.tensor_tensor(rhs[1][:, fc, :], c1, vx, OP.mult)
                    nc.gpsimd.tensor_tensor(rhs[2][:, fc, :], c1, vy, OP.mult)
                    nc.gpsimd.tensor_tensor(rhs[3][:, fc, :], c1, vz, OP.mult)

                # --- GEMM: y[comp] = rhs[comp] @ W (K=256 over fc chunks) ---
                stg = [stag.tile([128, 8, SUB], bf16, tag="stg", name=f"stg{si}_{j0}") for si in range(nsub)]
                ps = {}
                for comp in range(4):
                    for gc in range(2):
                        for si in range(nsub):
                            ps[(comp, gc, si)] = psum.tile([128, SUB], fp32, tag="ps", name=f"ps{comp}{gc}{si}_{j0}")

                for gc in range(2):
                    g0 = gc * 128
                    for fc in range(2):
                        for comp in range(4):
                            w_sb = w0_sb if comp == 0 else w1_sb
                            lhsT = w_sb[:, fc, g0 : g0 + 128]
                            for si in range(nsub):
                                o = si * SUB
                                nc.tensor.matmul(
                                    ps[(comp, gc, si)],
                                    lhsT,
                                    rhs[comp][:, fc, o : o + SUB],
                                    start=(fc == 0),
                                    stop=(fc == 1),
                                )
                                if fc == 1:
                                    nc.scalar.activation(
                                        stg[si][:, comp * 2 + gc, :],
                                        ps[(comp, gc, si)],
                                        AF.Copy,
                                    )

                for si in range(nsub):
                    o = j0 + si * SUB
                    nc.sync.dma_start(out=yr[:, :, o : o + SUB], in_=stg[si])

    nc.compile()
    return nc


def _prepare(node_feats, node_specie, w0, w1, W0, W1):
    """Host-side: sort by species, shard, transpose, fold scale factors."""
    n = node_feats.shape[0]
    sp = np.asarray(node_specie).astype(np.int64)

    ids_by_sp = [np.nonzero(sp == s)[0] for s in range(NUM_SPECIES)]
    core_ids = [[ids_by_sp[s][c::N_CORES] for s in range(NUM_SPECIES)] for c in range(N_CORES)]
    # even segment lengths keep bf16 slices 4B-aligned on device
    c_sp = [
        (max(len(core_ids[c][s]) for c in range(N_CORES)) + 1) // 2 * 2
        for s in range(NUM_SPECIES)
    ]
    ntot = int(np.sum(c_sp))
    pad_tail = (-ntot) % SUB
    c_sp[-1] += pad_tail
    ntot += pad_tail

    idx = np.zeros((N_CORES, ntot), dtype=np.int64)
    valid = np.zeros((N_CORES, ntot), dtype=bool)
    off = 0
    for s in range(NUM_SPECIES):
        L = c_sp[s]
        for c in range(N_CORES):
            ids = core_ids[c][s]
            k = len(ids)
            idx[c, off : off + k] = ids
            valid[c, off : off + k] = True
        off += L

    w0a = np.asarray(w0, np.float32).copy()
    w1a = np.asarray(w1, np.float32).copy()
    w0a[:, 2, :] *= INV_SQRT3
    w0a[:, 4, :] *= INV_SQRT3
    w1a[:, 3, :] *= INV_SQRT3
    a0_tab = np.ascontiguousarray(
        w0a.transpose(2, 0, 1).reshape(F, 5 * NUM_SPECIES)
    )
    a1_tab = np.ascontiguousarray(
        w1a.transpose(2, 0, 1).reshape(F, 4 * NUM_SPECIES)
    )
    W0s = (np.asarray(W0, np.float32) * INV_SQRT_F).astype(ml_dtypes.bfloat16)
    W1s = (np.asarray(W1, np.float32) * INV_SQRT_F).astype(ml_dtypes.bfloat16)

    xt = np.ascontiguousarray(
        np.asarray(node_feats, np.float32).transpose(2, 1, 0).astype(ml_dtypes.bfloat16)
    )  # [4,256,n] bf16
    xs = []
    for c in range(N_CORES):
        xc = xt[:, :, idx[c]]
        xs.append(np.ascontiguousarray(xc.reshape(8, 128, ntot)))

    return xs, idx, valid, tuple(c_sp), ntot, a0_tab, a1_tab, W0s, W1s


def kernel(node_feats, node_specie, w0, w1, W0, W1):
    from concourse.bass_utils import run_bass_kernel_spmd

    xs, idx, valid, c_sp, ntot, a0_tab, a1_tab, W0s, W1s = _prepare(
        node_feats, node_specie, w0, w1, W0, W1
    )

    key = (c_sp, ntot)
    if key not in _KERNEL_CACHE:
        _KERNEL_CACHE[key] = _build_bass(list(c_sp), ntot)
    nc = _KERNEL_CACHE[key]

    in_maps = [
        {"x": xs[c], "a0": a0_tab, "a1": a1_tab, "w0": W0s, "w1": W1s}
        for c in range(N_CORES)
    ]
    res = run_bass_kernel_spmd(nc, in_maps, core_ids=list(range(N_CORES)))

    n = node_feats.shape[0]
    out = np.empty((n, F, 4), dtype=np.float32)
    for c in range(N_CORES):
        yc = res.results[c]["y"].reshape(4, F, ntot).astype(np.float32)
        yt = np.ascontiguousarray(yc.transpose(2, 1, 0))
        m = valid[c]
        out[idx[c][m]] = yt[m]
    return out
